# revision 3
# baseline (speedup 1.0000x reference)
"""Trainium2 Bass kernel for nn_Meta_Graph1_40114994545303 (gnn_message_passing).

Math: the reference returns only the global-node row of the GCN output.
With mask = (attribute_label > 0), star adjacency means
    out[s, :] = tanh( (sum_a mask[s,a] * attribute_feat[s,a,:]) @ W + b )
and x never reaches the output (adj[A, A] = 0).

Two implementations:

ws (default) -- W column-sharded across the 8 cores + AllGather of the
  masked sums. Per core:
    stage 1 (own 32 samples): masked sum as block-diag-mask matmul,
      feat moving / mask stationary -> psum [4x32, 512].
    exchange: psum -> sbuf -> HBM bounce [32, 2048] f16, AllGather ->
      ms_all [256, 2048] f16 (Shared), then DMA-transpose reads
      ms_all -> msT_sb [128, 16, 256] (contraction dim onto partitions).
    stage 2: out[:, cols_c] = tanh(ms_all @ W[:, cols_c] + b[cols_c]) as
      16 K-chunks x 2 sample-groups of [128, 256] matmuls; W shard is only
      1 MB f16 (vs 8 MB replicated). tanh on scalar engine, out [256, 256].
  Host assembles np.concatenate(outs, axis=1).

dp (fallback) -- original fully data-parallel fp16 kernel (W replicated).
"""

import os
from contextlib import ExitStack

import numpy as np

import concourse.bacc as bacc
import concourse.mybir as mybir

B, A, D = 256, 32, 2048
NCORES = 8
S = B // NCORES  # 32 samples per core
P = 128
NS = D // NCORES  # 256 output columns per core (ws)
KC1 = (S * A) // P  # 8 k-chunks in stage 1
KC2 = D // P  # 16 k-chunks in stage 2
NT = D // 512  # 4 psum column tiles (stage 1 / dp stage 2)
F32 = mybir.dt.float32
F16 = mybir.dt.float16


def build_nc_ws(warmup: int = 0):
    cf = 4  # feat k-chunks per DMA
    NF = KC1 // cf
    nc = bacc.Bacc("TRN2", target_bir_lowering=False, debug=False)

    feat = nc.dram_tensor("feat", [S * A, D], F16, kind="ExternalInput")
    mbdt = nc.dram_tensor("mbdt", [P, KC1 * S], F16, kind="ExternalInput")
    wsh = nc.dram_tensor("wsh", [D, NS], F16, kind="ExternalInput")
    bias = nc.dram_tensor("bias", [1, NS], F16, kind="ExternalInput")
    onesd = nc.dram_tensor("ones", [1, P], F16, kind="ExternalInput")
    out = nc.dram_tensor("out", [B, NS], F32, kind="ExternalOutput")
    ms_bounce = nc.dram_tensor("ms_bounce", [S, D], F16)
    ms_all = nc.dram_tensor("ms_all", [B, D], F16, addr_space="Shared")

    with ExitStack() as ctx:
        feat_sb = ctx.enter_context(nc.sbuf_tensor([P, KC1, D], F16))
        wsh_sb = ctx.enter_context(nc.sbuf_tensor([P, KC2, NS], F16))
        mbdt_sb = ctx.enter_context(nc.sbuf_tensor([P, KC1, S], F16))
        bias_sb = ctx.enter_context(nc.sbuf_tensor([1, NS], F16))
        ones_sb = ctx.enter_context(nc.sbuf_tensor([1, P], F16))
        msc_sb = ctx.enter_context(nc.sbuf_tensor([P, 512], F16))
        msT_sb = ctx.enter_context(nc.sbuf_tensor([P, KC2, B], F16))
        out_sb = ctx.enter_context(nc.sbuf_tensor([P, 2 * NS], F32))
        pm_bank = ctx.enter_context(nc.psum_tensor([P, 512], F32))
        po_bank = ctx.enter_context(nc.psum_tensor([P, 512], F32))
        pw_bank = ctx.enter_context(nc.psum_tensor([P, 128], F32))
        fsems = [ctx.enter_context(nc.semaphore(f"fs{g}")) for g in range(NF)]
        csem = ctx.enter_context(nc.semaphore("csem"))
        wsem = ctx.enter_context(nc.semaphore("wsem"))
        s1_sem = ctx.enter_context(nc.semaphore("s1_sem"))
        cpy_sem = ctx.enter_context(nc.semaphore("cpy_sem"))
        bnc_sem = ctx.enter_context(nc.semaphore("bnc_sem"))
        cc_sem = ctx.enter_context(nc.semaphore("cc_sem"))
        tr_sems = [ctx.enter_context(nc.semaphore(f"tr{g}")) for g in range(2)]
        s2_sems = [ctx.enter_context(nc.semaphore(f"s2g{g}")) for g in range(2)]
        act_sems = [ctx.enter_context(nc.semaphore(f"act{g}")) for g in range(2)]
        osem = ctx.enter_context(nc.semaphore("osem"))
        block = ctx.enter_context(nc.Block(no_gpsimd_drain=True))

        @block.sync
        def _(sync):
            for g in range(NF):
                sync.dma_start(
                    feat_sb[:, g * cf : (g + 1) * cf, :],
                    feat[g * cf * P : (g + 1) * cf * P, :].rearrange(
                        "(c p) d -> p c d", p=P
                    ),
                ).then_inc(fsems[g], 16)
            # psum->sbuf copy done by vector; then bounce to HBM
            sync.wait_ge(cpy_sem, 1)
            sync.dma_start(
                ms_bounce[:, :].rearrange("s (g u) -> g s u", g=NT),
                msc_sb[:, :].rearrange("(g s) u -> g s u", g=NT),
            ).then_inc(bnc_sem, 16)
            # after the AllGather: transposed reads, one per sample-group
            sync.wait_ge(cc_sem, 1)
            for grp in range(2):
                sync.dma_start(
                    msT_sb[:, :, grp * P : (grp + 1) * P],
                    ms_all[grp * P : (grp + 1) * P, :],
                    transpose=True,
                ).then_inc(tr_sems[grp], 16)
            for grp in range(2):
                sync.wait_ge(act_sems[grp], 1)
                sync.dma_start(
                    out[grp * P : (grp + 1) * P, :],
                    out_sb[:, grp * NS : (grp + 1) * NS],
                ).then_inc(osem, 16)
            sync.wait_ge(osem, 32)

        @block.scalar
        def _(scalar):
            scalar.dma_start(
                mbdt_sb[:], mbdt[:].rearrange("p (k j) -> p k j", k=KC1)
            ).then_inc(csem, 16)
            scalar.dma_start(bias_sb[:], bias[:]).then_inc(csem, 16)
            scalar.dma_start(ones_sb[:], onesd[:]).then_inc(csem, 16)
            scalar.dma_start(
                wsh_sb[:], wsh[:, :].rearrange("(k p) n -> p k n", p=P)
            ).then_inc(wsem, 16)
            for grp in range(2):
                scalar.wait_ge(s2_sems[grp], 1)
                nc.scalar.activation(
                    out_sb[:, grp * NS : (grp + 1) * NS],
                    po_bank[:, grp * NS : (grp + 1) * NS],
                    mybir.ActivationFunctionType.Tanh,
                ).then_inc(act_sems[grp], 1)

        @block.vector
        def _(vector):
            vector.wait_ge(s1_sem, 1)
            nc.vector.tensor_copy(msc_sb[:], pm_bank[:]).then_inc(cpy_sem, 1)

        @block.gpsimd
        def _(gpsimd):
            gpsimd.wait_ge(bnc_sem, 16)
            gpsimd.collective_compute(
                "AllGather",
                mybir.AluOpType.bypass,
                replica_groups=[list(range(NCORES))],
                ins=[ms_bounce.ap().opt()],
                outs=[ms_all.ap().opt()],
            ).then_inc(cc_sem)

        @block.tensor
        def _(tensor):
            tensor.wait_ge(csem, 48)
            # bias as the first accumulation of both stage-2 groups
            for grp in range(2):
                nc.tensor.matmul(
                    po_bank[:, grp * NS : (grp + 1) * NS],
                    ones_sb[:],
                    bias_sb[:],
                    start=True,
                    stop=False,
                    skip_group_check=True,
                )
            last = None
            for g in range(NF):
                tensor.wait_ge(fsems[g], 16)
                for c in range(cf):
                    k = g * cf + c
                    for n in range(NT):
                        last = nc.tensor.matmul(
                            pm_bank[n * S : (n + 1) * S, :],
                            mbdt_sb[:, k, :],
                            feat_sb[:, k, n * 512 : (n + 1) * 512],
                            start=(k == 0),
                            stop=(k == KC1 - 1),
                            tile_position=(0, n * S),
                            skip_group_check=True,
                        )
            last.then_inc(s1_sem, 1)
            # keep the PE HAM-warm through the AllGather wait
            for i in range(warmup):
                nc.tensor.matmul(
                    pw_bank[:, :],
                    mbdt_sb[:, 0, :],
                    mbdt_sb[:, 0, :],
                    start=True,
                    stop=True,
                    skip_group_check=True,
                )
            tensor.wait_ge(wsem, 16)
            for grp in range(2):
                tensor.wait_ge(tr_sems[grp], 16)
                lastb = None
                for k2 in range(KC2):
                    lastb = nc.tensor.matmul(
                        po_bank[:, grp * NS : (grp + 1) * NS],
                        msT_sb[:, k2, grp * P : (grp + 1) * P],
                        wsh_sb[:, k2, :],
                        start=False,
                        stop=(k2 == KC2 - 1),
                        skip_group_check=True,
                    )
                lastb.then_inc(s2_sems[grp], 1)

    nc.compile()
    return nc


def build_nc_dp():
    """Original data-parallel fp16 kernel (W replicated), known-good."""
    cdt = F16
    cf = 4
    WCH = [4, 4, 4, 4]
    WST = [0, 4, 8, 12]
    NF, NW = KC1 // cf, len(WCH)
    nc = bacc.Bacc("TRN2", target_bir_lowering=False, debug=False)

    feat = nc.dram_tensor("feat", [S * A, D], cdt, kind="ExternalInput")
    mbdt = nc.dram_tensor("mbdt", [P, KC1 * S], cdt, kind="ExternalInput")
    w = nc.dram_tensor("w", [D, D], cdt, kind="ExternalInput")
    bias = nc.dram_tensor("bias", [1, D], cdt, kind="ExternalInput")
    onesd = nc.dram_tensor("ones", [1, S], cdt, kind="ExternalInput")
    out = nc.dram_tensor("out", [S, D], F32, kind="ExternalOutput")

    with ExitStack() as ctx:
        feat_sb = ctx.enter_context(nc.sbuf_tensor([P, KC1, D], cdt))
        w_sb = ctx.enter_context(nc.sbuf_tensor([P, KC2, D], cdt))
        mbdt_sb = ctx.enter_context(nc.sbuf_tensor([P, KC1, S], cdt))
        bias_sb = ctx.enter_context(nc.sbuf_tensor([1, D], cdt))
        ones_sb = ctx.enter_context(nc.sbuf_tensor([1, S], cdt))
        msc_sb = ctx.enter_context(nc.sbuf_tensor([P, 512], cdt))
        msT_sb = ctx.enter_context(nc.sbuf_tensor([P, KC2, S], cdt))
        out_sb = ctx.enter_context(nc.sbuf_tensor([P, 512], F32))
        pm_bank = ctx.enter_context(nc.psum_tensor([P, 512], F32))
        po_bank = ctx.enter_context(nc.psum_tensor([P, 512], F32))
        fsems = [ctx.enter_context(nc.semaphore(f"fs{g}")) for g in range(NF)]
        wsems = [ctx.enter_context(nc.semaphore(f"ws{g}")) for g in range(NW)]
        csem = ctx.enter_context(nc.semaphore("csem"))
        osem = ctx.enter_context(nc.semaphore("osem"))
        s1_sem = ctx.enter_context(nc.semaphore("s1_sem"))
        tr_sem = ctx.enter_context(nc.semaphore("tr_sem"))
        s2_sem = ctx.enter_context(nc.semaphore("s2_sem"))
        act_sem = ctx.enter_context(nc.semaphore("act_sem"))
        osem2 = ctx.enter_context(nc.semaphore("osem2"))
        block = ctx.enter_context(nc.Block(no_gpsimd_drain=True))

        @block.sync
        def _(sync):
            for g in range(NF):
                sync.dma_start(
                    feat_sb[:, g * cf : (g + 1) * cf, :],
                    feat[g * cf * P : (g + 1) * cf * P, :].rearrange(
                        "(c p) d -> p c d", p=P
                    ),
                ).then_inc(fsems[g], 16)
            for g in range(NW):
                st, ln = WST[g], WCH[g]
                sync.dma_start(
                    w_sb[:, st : st + ln, :],
                    w[st * P : (st + ln) * P, :].rearrange("(c p) d -> p c d", p=P),
                ).then_inc(wsems[g], 16)
            sync.wait_ge(act_sem, 1)
            for n in (0, 2):
                sync.dma_start(
                    out[:, n * 512 : (n + 1) * 512], out_sb[n * S : (n + 1) * S, :]
                ).then_inc(osem2, 16)
            sync.wait_ge(osem2, 32)

        @block.scalar
        def _(scalar):
            scalar.dma_start(
                mbdt_sb[:], mbdt[:].rearrange("p (k j) -> p k j", k=KC1)
            ).then_inc(csem, 16)
            scalar.dma_start(bias_sb[:], bias[:]).then_inc(csem, 16)
            scalar.dma_start(ones_sb[:], onesd[:]).then_inc(csem, 16)
            scalar.wait_ge(s2_sem, 1)
            nc.scalar.activation(
                out_sb[:], po_bank[:], mybir.ActivationFunctionType.Tanh
            ).then_inc(act_sem, 1)
            scalar.wait_ge(act_sem, 1)
            for n in (1, 3):
                scalar.dma_start(
                    out[:, n * 512 : (n + 1) * 512], out_sb[n * S : (n + 1) * S, :]
                ).then_inc(osem, 16)
            scalar.wait_ge(osem, 32)

        @block.vector
        def _(vector):
            vector.wait_ge(s1_sem, 1)
            nc.vector.tensor_copy(msc_sb[:], pm_bank[:])
            nc.vector.drain()
            last = None
            for n in range(NT):
                for q in range(512 // 32):
                    d0 = n * 512 + q * 32
                    k2, r = divmod(d0, P)
                    last = nc.vector.transpose(
                        msT_sb[r : r + 32, k2, :],
                        msc_sb[n * S : (n + 1) * S, q * 32 : (q + 1) * 32],
                    )
            last.then_inc(tr_sem, 1)

        @block.tensor
        def _(tensor):
            tensor.wait_ge(csem, 48)
            for n in range(NT):
                nc.tensor.matmul(
                    po_bank[n * S : (n + 1) * S, :],
                    ones_sb[:],
                    bias_sb[:, n * 512 : (n + 1) * 512],
                    start=True,
                    stop=False,
                    tile_position=(0, n * S),
                    skip_group_check=True,
                )
            last = None
            for g in range(NF):
                tensor.wait_ge(fsems[g], 16)
                for c in range(cf):
                    k = g * cf + c
                    for n in range(NT):
                        last = nc.tensor.matmul(
                            pm_bank[n * S : (n + 1) * S, :],
                            mbdt_sb[:, k, :],
                            feat_sb[:, k, n * 512 : (n + 1) * 512],
                            start=(k == 0),
                            stop=(k == KC1 - 1),
                            tile_position=(0, n * S),
                            skip_group_check=True,
                        )
            last.then_inc(s1_sem, 1)
            tensor.wait_ge(tr_sem, 1)
            lastb = None
            for g in range(NW):
                tensor.wait_ge(wsems[g], 16)
                for c in range(WCH[g]):
                    k2 = WST[g] + c
                    for n in range(NT):
                        lastb = nc.tensor.matmul(
                            po_bank[n * S : (n + 1) * S, :],
                            msT_sb[:, k2, :],
                            w_sb[:, k2, n * 512 : (n + 1) * 512],
                            start=False,
                            stop=(k2 == KC2 - 1),
                            tile_position=(0, n * S),
                            skip_group_check=True,
                        )
            lastb.then_inc(s2_sem, 1)

    nc.compile()
    return nc


def _mbd_blockdiag(mask_c: np.ndarray) -> np.ndarray:
    """mask_c [S, A] -> block-diag [P, KC1*S] fp16 device layout."""
    mbd = np.zeros((KC1, P, S), np.float32)
    for k in range(KC1):
        for sl in range(P // A):
            smp = (P // A) * k + sl
            mbd[k, sl * A : (sl + 1) * A, smp] = mask_c[smp]
    return np.ascontiguousarray(mbd.transpose(1, 0, 2)).reshape(P, KC1 * S)


def _host_prep_ws(inputs: dict):
    feat = np.asarray(inputs["attribute_feat"], dtype=np.float32)
    label = np.asarray(inputs["attribute_label"])
    w = np.asarray(inputs["W"], dtype=np.float32).astype(np.float16)
    b = np.asarray(inputs["b"], dtype=np.float32).astype(np.float16).reshape(1, D)
    mask = (label > 0).astype(np.float32)

    in_maps = []
    for c in range(NCORES):
        in_maps.append(
            {
                "feat": feat[c * S : (c + 1) * S].reshape(S * A, D).astype(np.float16),
                "mbdt": _mbd_blockdiag(mask[c * S : (c + 1) * S]).astype(np.float16),
                "wsh": np.ascontiguousarray(w[:, c * NS : (c + 1) * NS]),
                "bias": np.ascontiguousarray(b[:, c * NS : (c + 1) * NS]),
                "ones": np.ones((1, P), np.float16),
            }
        )
    return in_maps


def _host_prep_dp(inputs: dict):
    feat = np.asarray(inputs["attribute_feat"], dtype=np.float32)
    label = np.asarray(inputs["attribute_label"])
    w = np.asarray(inputs["W"], dtype=np.float32).astype(np.float16)
    b = np.asarray(inputs["b"], dtype=np.float32).astype(np.float16).reshape(1, D)
    mask = (label > 0).astype(np.float32)

    in_maps = []
    for c in range(NCORES):
        in_maps.append(
            {
                "feat": feat[c * S : (c + 1) * S].reshape(S * A, D).astype(np.float16),
                "mbdt": _mbd_blockdiag(mask[c * S : (c + 1) * S]).astype(np.float16),
                "w": w,
                "bias": b,
                "ones": np.ones((1, S), np.float16),
            }
        )
    return in_maps


_NC_CACHE: dict = {}


def run(inputs: dict, compute_dtype: str = "fp16", trace: bool = False):
    from concourse.bass_utils import run_bass_kernel_spmd

    impl = os.environ.get("GNN_KERNEL_IMPL", "ws")
    if impl not in _NC_CACHE:
        if impl == "ws":
            _NC_CACHE[impl] = build_nc_ws(
                warmup=int(os.environ.get("GNN_WARMUP", "0"))
            )
        else:
            _NC_CACHE[impl] = build_nc_dp()
    nc = _NC_CACHE[impl]
    if impl == "ws":
        in_maps = _host_prep_ws(inputs)
        res = run_bass_kernel_spmd(nc, in_maps, list(range(NCORES)), trace=trace)
        out = np.concatenate(
            [res.results[c]["out"] for c in range(NCORES)], axis=1
        ).astype(np.float32)
    else:
        in_maps = _host_prep_dp(inputs)
        res = run_bass_kernel_spmd(nc, in_maps, list(range(NCORES)), trace=trace)
        out = np.concatenate(
            [res.results[c]["out"] for c in range(NCORES)], axis=0
        ).astype(np.float32)
    return out, res


def kernel(**inputs) -> np.ndarray:
    out, _ = run(inputs)
    return out


# revision 5
# speedup vs baseline: 2.5527x; 2.5527x over previous
"""Trainium2 Bass kernel for nn_Meta_Graph1_40114994545303 (gnn_message_passing).

Math: the reference returns only the global-node row of the GCN output.
With mask = (attribute_label > 0), star adjacency means
    out[s, :] = tanh( (sum_a mask[s,a] * attribute_feat[s,a,:]) @ W + b )
and x never reaches the output (adj[A, A] = 0).

Data-parallel over batch: 32 samples per core on 8 cores, W replicated (f16).
Per core:
  stage 1: masked sum over attributes. The host packs only the masked rows
    (mask=1) of attribute_feat contiguously (zero-padded to K1P*128 rows),
    so the feat stream carries ~half the bytes; a host-built block-select
    matrix (ones at [row -> sample]) is the stationary operand and the
    packed feat rows stream through. Four serial 512-col matmuls per
    k-chunk into four psum banks at partitions 0:32 (sample-aligned).
  transpose: psum -> msc_nat [32, 2048] f16 (4 DVE copies, same partitions),
    then ONE SBUF->SBUF DMA-transpose to msT [128, 16, 32] (d on partitions).
  stage 2: [32, 2048] @ W as 16 K-chunk matmuls, msT chunks stationary and
    W streamed 512-col col-tiled x4; bias folded in as a rank-1 matmul of
    ones x b into the same PSUM accumulation; tanh on scalar engine.

Fallback `dp8` (full unpacked feat, DVE 32x32 transposes) is compiled only
if an input exceeds the packed-row capacity.
"""

import os
from contextlib import ExitStack

import numpy as np

import concourse.bacc as bacc
import concourse.mybir as mybir

B, A, D = 256, 32, 2048
NCORES = 8
S = B // NCORES  # 32 samples per core
P = 128
KC1 = (S * A) // P  # 8 k-chunks of unpacked feat
K1P = 5  # packed-feat k-chunks (640 rows; P(Binom(1024,.5) > 640) ~ 6e-16)
KC2 = D // P  # 16 k-chunks in stage 2
NT = D // 512  # 4 psum-bank column tiles
F32 = mybir.dt.float32
F16 = mybir.dt.float16


def build_nc_packed():
    WCH = [4, 4, 4, 2, 2]  # W transfer sizes (k2-chunks); short tail
    WST = [0, 4, 8, 12, 14]
    FCH = [3, 2]  # packed-feat transfer sizes (k1-chunks)
    FST = [0, 3]
    NF, NW = len(FCH), len(WCH)
    nc = bacc.Bacc("TRN2", target_bir_lowering=False, debug=False)

    feat = nc.dram_tensor("feat", [K1P * P, D], F16, kind="ExternalInput")
    mbdt = nc.dram_tensor("mbdt", [P, K1P * S], F16, kind="ExternalInput")
    w = nc.dram_tensor("w", [D, D], F16, kind="ExternalInput")
    bias = nc.dram_tensor("bias", [1, D], F16, kind="ExternalInput")
    onesd = nc.dram_tensor("ones", [1, S], F16, kind="ExternalInput")
    out = nc.dram_tensor("out", [S, D], F32, kind="ExternalOutput")

    with ExitStack() as ctx:
        feat_sb = ctx.enter_context(nc.sbuf_tensor([P, K1P, D], F16))
        w_sb = ctx.enter_context(nc.sbuf_tensor([P, KC2, D], F16))
        mbdt_sb = ctx.enter_context(nc.sbuf_tensor([P, K1P, S], F16))
        bias_sb = ctx.enter_context(nc.sbuf_tensor([1, D], F16))
        ones_sb = ctx.enter_context(nc.sbuf_tensor([1, S], F16))
        msc_sb = ctx.enter_context(nc.sbuf_tensor([S, D], F16))
        msT_sb = ctx.enter_context(nc.sbuf_tensor([P, KC2, S], F16))
        out_sb = ctx.enter_context(nc.sbuf_tensor([P, 512], F32))
        pm_banks = [
            ctx.enter_context(nc.psum_tensor(f"pm{n}", [P, 512], F32))
            for n in range(NT)
        ]
        po_bank = ctx.enter_context(nc.psum_tensor([P, 512], F32))
        fsems = [ctx.enter_context(nc.semaphore(f"fs{g}")) for g in range(NF)]
        wsems = [ctx.enter_context(nc.semaphore(f"ws{g}")) for g in range(NW)]
        csem = ctx.enter_context(nc.semaphore("csem"))
        osem = ctx.enter_context(nc.semaphore("osem"))
        s1_sem = ctx.enter_context(nc.semaphore("s1_sem"))
        cp_sem = ctx.enter_context(nc.semaphore("cp_sem"))
        tr_sem = ctx.enter_context(nc.semaphore("tr_sem"))
        s2_sem = ctx.enter_context(nc.semaphore("s2_sem"))
        act_sem = ctx.enter_context(nc.semaphore("act_sem"))
        osem2 = ctx.enter_context(nc.semaphore("osem2"))
        block = ctx.enter_context(nc.Block(no_gpsimd_drain=True))

        @block.sync
        def _(sync):
            for g in range(NF):
                st, ln = FST[g], FCH[g]
                sync.dma_start(
                    feat_sb[:, st : st + ln, :],
                    feat[st * P : (st + ln) * P, :].rearrange(
                        "(c p) d -> p c d", p=P
                    ),
                ).then_inc(fsems[g], 16)
            for g in range(NW):
                st, ln = WST[g], WCH[g]
                sync.dma_start(
                    w_sb[:, st : st + ln, :],
                    w[st * P : (st + ln) * P, :].rearrange("(c p) d -> p c d", p=P),
                ).then_inc(wsems[g], 16)
            sync.wait_ge(act_sem, 1)
            for n in (0, 2):
                sync.dma_start(
                    out[:, n * 512 : (n + 1) * 512], out_sb[n * S : (n + 1) * S, :]
                ).then_inc(osem2, 16)
            sync.wait_ge(osem2, 32)

        @block.scalar
        def _(scalar):
            scalar.dma_start(
                mbdt_sb[:], mbdt[:].rearrange("p (k j) -> p k j", k=K1P)
            ).then_inc(csem, 16)
            scalar.dma_start(bias_sb[:], bias[:]).then_inc(csem, 16)
            scalar.dma_start(ones_sb[:], onesd[:]).then_inc(csem, 16)
            # the one SBUF->SBUF transpose: msc_nat [32, 2048] -> [128, 16, 32]
            scalar.wait_ge(cp_sem, NT)
            scalar.dma_start(msT_sb[:, :, :], msc_sb[:, :], transpose=True).then_inc(
                tr_sem, 16
            )
            scalar.wait_ge(s2_sem, 1)
            nc.scalar.activation(
                out_sb[:], po_bank[:], mybir.ActivationFunctionType.Tanh
            ).then_inc(act_sem, 1)
            scalar.wait_ge(act_sem, 1)
            for n in (1, 3):
                scalar.dma_start(
                    out[:, n * 512 : (n + 1) * 512], out_sb[n * S : (n + 1) * S, :]
                ).then_inc(osem, 16)
            scalar.wait_ge(osem, 32)

        @block.vector
        def _(vector):
            vector.wait_ge(s1_sem, 1)
            last = None
            for n in range(NT):
                last = nc.vector.tensor_copy(
                    msc_sb[:, n * 512 : (n + 1) * 512], pm_banks[n][0:S, :]
                )
            for n in range(NT):
                pass
            last.then_inc(cp_sem, NT)

        @block.tensor
        def _(tensor):
            tensor.wait_ge(csem, 48)
            for n in range(NT):
                nc.tensor.matmul(
                    po_bank[n * S : (n + 1) * S, :],
                    ones_sb[:],
                    bias_sb[:, n * 512 : (n + 1) * 512],
                    start=True,
                    stop=False,
                    tile_position=(0, n * S),
                    skip_group_check=True,
                )
            last = None
            for g in range(NF):
                tensor.wait_ge(fsems[g], 16)
                for c in range(FCH[g]):
                    k = FST[g] + c
                    for n in range(NT):
                        last = nc.tensor.matmul(
                            pm_banks[n][0:S, :],
                            mbdt_sb[:, k, :],
                            feat_sb[:, k, n * 512 : (n + 1) * 512],
                            start=(k == 0),
                            stop=(k == K1P - 1),
                            skip_group_check=True,
                        )
            last.then_inc(s1_sem, 1)
            tensor.wait_ge(tr_sem, 16)
            lastb = None
            for g in range(NW):
                tensor.wait_ge(wsems[g], 16)
                for c in range(WCH[g]):
                    k2 = WST[g] + c
                    for n in range(NT):
                        lastb = nc.tensor.matmul(
                            po_bank[n * S : (n + 1) * S, :],
                            msT_sb[:, k2, :],
                            w_sb[:, k2, n * 512 : (n + 1) * 512],
                            start=False,
                            stop=(k2 == KC2 - 1),
                            tile_position=(0, n * S),
                            skip_group_check=True,
                        )
            lastb.then_inc(s2_sem, 1)

    nc.compile()
    return nc


def build_nc_dp8():
    """Fallback: full unpacked feat (KC1=8), DVE 32x32 transposes."""
    cdt = F16
    cf = 4
    WCH = [4, 4, 4, 4]
    WST = [0, 4, 8, 12]
    NF, NW = KC1 // cf, len(WCH)
    nc = bacc.Bacc("TRN2", target_bir_lowering=False, debug=False)

    feat = nc.dram_tensor("feat", [S * A, D], cdt, kind="ExternalInput")
    mbdt = nc.dram_tensor("mbdt", [P, KC1 * S], cdt, kind="ExternalInput")
    w = nc.dram_tensor("w", [D, D], cdt, kind="ExternalInput")
    bias = nc.dram_tensor("bias", [1, D], cdt, kind="ExternalInput")
    onesd = nc.dram_tensor("ones", [1, S], cdt, kind="ExternalInput")
    out = nc.dram_tensor("out", [S, D], F32, kind="ExternalOutput")

    with ExitStack() as ctx:
        feat_sb = ctx.enter_context(nc.sbuf_tensor([P, KC1, D], cdt))
        w_sb = ctx.enter_context(nc.sbuf_tensor([P, KC2, D], cdt))
        mbdt_sb = ctx.enter_context(nc.sbuf_tensor([P, KC1, S], cdt))
        bias_sb = ctx.enter_context(nc.sbuf_tensor([1, D], cdt))
        ones_sb = ctx.enter_context(nc.sbuf_tensor([1, S], cdt))
        msc_sb = ctx.enter_context(nc.sbuf_tensor([P, 512], cdt))
        msT_sb = ctx.enter_context(nc.sbuf_tensor([P, KC2, S], cdt))
        out_sb = ctx.enter_context(nc.sbuf_tensor([P, 512], F32))
        pm_bank = ctx.enter_context(nc.psum_tensor([P, 512], F32))
        po_bank = ctx.enter_context(nc.psum_tensor([P, 512], F32))
        fsems = [ctx.enter_context(nc.semaphore(f"fs{g}")) for g in range(NF)]
        wsems = [ctx.enter_context(nc.semaphore(f"ws{g}")) for g in range(NW)]
        csem = ctx.enter_context(nc.semaphore("csem"))
        osem = ctx.enter_context(nc.semaphore("osem"))
        s1_sem = ctx.enter_context(nc.semaphore("s1_sem"))
        tr_sem = ctx.enter_context(nc.semaphore("tr_sem"))
        s2_sem = ctx.enter_context(nc.semaphore("s2_sem"))
        act_sem = ctx.enter_context(nc.semaphore("act_sem"))
        osem2 = ctx.enter_context(nc.semaphore("osem2"))
        block = ctx.enter_context(nc.Block(no_gpsimd_drain=True))

        @block.sync
        def _(sync):
            for g in range(NF):
                sync.dma_start(
                    feat_sb[:, g * cf : (g + 1) * cf, :],
                    feat[g * cf * P : (g + 1) * cf * P, :].rearrange(
                        "(c p) d -> p c d", p=P
                    ),
                ).then_inc(fsems[g], 16)
            for g in range(NW):
                st, ln = WST[g], WCH[g]
                sync.dma_start(
                    w_sb[:, st : st + ln, :],
                    w[st * P : (st + ln) * P, :].rearrange("(c p) d -> p c d", p=P),
                ).then_inc(wsems[g], 16)
            sync.wait_ge(act_sem, 1)
            for n in (0, 2):
                sync.dma_start(
                    out[:, n * 512 : (n + 1) * 512], out_sb[n * S : (n + 1) * S, :]
                ).then_inc(osem2, 16)
            sync.wait_ge(osem2, 32)

        @block.scalar
        def _(scalar):
            scalar.dma_start(
                mbdt_sb[:], mbdt[:].rearrange("p (k j) -> p k j", k=KC1)
            ).then_inc(csem, 16)
            scalar.dma_start(bias_sb[:], bias[:]).then_inc(csem, 16)
            scalar.dma_start(ones_sb[:], onesd[:]).then_inc(csem, 16)
            scalar.wait_ge(s2_sem, 1)
            nc.scalar.activation(
                out_sb[:], po_bank[:], mybir.ActivationFunctionType.Tanh
            ).then_inc(act_sem, 1)
            scalar.wait_ge(act_sem, 1)
            for n in (1, 3):
                scalar.dma_start(
                    out[:, n * 512 : (n + 1) * 512], out_sb[n * S : (n + 1) * S, :]
                ).then_inc(osem, 16)
            scalar.wait_ge(osem, 32)

        @block.vector
        def _(vector):
            vector.wait_ge(s1_sem, 1)
            nc.vector.tensor_copy(msc_sb[:], pm_bank[:])
            nc.vector.drain()
            last = None
            for n in range(NT):
                for q in range(512 // 32):
                    d0 = n * 512 + q * 32
                    k2, r = divmod(d0, P)
                    last = nc.vector.transpose(
                        msT_sb[r : r + 32, k2, :],
                        msc_sb[n * S : (n + 1) * S, q * 32 : (q + 1) * 32],
                    )
            last.then_inc(tr_sem, 1)

        @block.tensor
        def _(tensor):
            tensor.wait_ge(csem, 48)
            for n in range(NT):
                nc.tensor.matmul(
                    po_bank[n * S : (n + 1) * S, :],
                    ones_sb[:],
                    bias_sb[:, n * 512 : (n + 1) * 512],
                    start=True,
                    stop=False,
                    tile_position=(0, n * S),
                    skip_group_check=True,
                )
            last = None
            for g in range(NF):
                tensor.wait_ge(fsems[g], 16)
                for c in range(cf):
                    k = g * cf + c
                    for n in range(NT):
                        last = nc.tensor.matmul(
                            pm_bank[n * S : (n + 1) * S, :],
                            mbdt_sb[:, k, :],
                            feat_sb[:, k, n * 512 : (n + 1) * 512],
                            start=(k == 0),
                            stop=(k == KC1 - 1),
                            tile_position=(0, n * S),
                            skip_group_check=True,
                        )
            last.then_inc(s1_sem, 1)
            tensor.wait_ge(tr_sem, 1)
            lastb = None
            for g in range(NW):
                tensor.wait_ge(wsems[g], 16)
                for c in range(WCH[g]):
                    k2 = WST[g] + c
                    for n in range(NT):
                        lastb = nc.tensor.matmul(
                            po_bank[n * S : (n + 1) * S, :],
                            msT_sb[:, k2, :],
                            w_sb[:, k2, n * 512 : (n + 1) * 512],
                            start=False,
                            stop=(k2 == KC2 - 1),
                            tile_position=(0, n * S),
                            skip_group_check=True,
                        )
            lastb.then_inc(s2_sem, 1)

    nc.compile()
    return nc


def _mbd_blockdiag_unpacked(mask_c: np.ndarray) -> np.ndarray:
    mbd = np.zeros((KC1, P, S), np.float32)
    for k in range(KC1):
        for sl in range(P // A):
            smp = (P // A) * k + sl
            mbd[k, sl * A : (sl + 1) * A, smp] = mask_c[smp]
    return np.ascontiguousarray(mbd.transpose(1, 0, 2)).reshape(P, KC1 * S)


def _host_prep_packed(inputs: dict):
    feat = np.asarray(inputs["attribute_feat"], dtype=np.float32)
    label = np.asarray(inputs["attribute_label"])
    w16 = np.asarray(inputs["W"], dtype=np.float32).astype(np.float16)
    b = np.asarray(inputs["b"], dtype=np.float32).astype(np.float16).reshape(1, D)
    mask = np.asarray(label) > 0

    in_maps = []
    for c in range(NCORES):
        m_c = mask[c * S : (c + 1) * S]  # [S, A] bool
        f_c = feat[c * S : (c + 1) * S]  # [S, A, D]
        smp_idx, att_idx = np.nonzero(m_c)
        nrows = len(smp_idx)
        if nrows > K1P * P:
            return None  # overflow: caller falls back to dp8
        packed = np.zeros((K1P * P, D), np.float16)
        packed[:nrows] = f_c[smp_idx, att_idx].astype(np.float16)
        sel = np.zeros((K1P * P, S), np.float32)
        sel[np.arange(nrows), smp_idx] = 1.0
        # device layout [P, (k, s)]
        sel_dev = np.ascontiguousarray(
            sel.reshape(K1P, P, S).transpose(1, 0, 2)
        ).reshape(P, K1P * S)
        in_maps.append(
            {
                "feat": packed,
                "mbdt": sel_dev.astype(np.float16),
                "w": w16,
                "bias": b,
                "ones": np.ones((1, S), np.float16),
            }
        )
    return in_maps


def _host_prep_dp8(inputs: dict):
    feat = np.asarray(inputs["attribute_feat"], dtype=np.float32)
    label = np.asarray(inputs["attribute_label"])
    w16 = np.asarray(inputs["W"], dtype=np.float32).astype(np.float16)
    b = np.asarray(inputs["b"], dtype=np.float32).astype(np.float16).reshape(1, D)
    mask = (np.asarray(label) > 0).astype(np.float32)
    in_maps = []
    for c in range(NCORES):
        in_maps.append(
            {
                "feat": feat[c * S : (c + 1) * S]
                .reshape(S * A, D)
                .astype(np.float16),
                "mbdt": _mbd_blockdiag_unpacked(mask[c * S : (c + 1) * S]).astype(
                    np.float16
                ),
                "w": w16,
                "bias": b,
                "ones": np.ones((1, S), np.float16),
            }
        )
    return in_maps


_NC_CACHE: dict = {}


def run(inputs: dict, compute_dtype: str = "fp16", trace: bool = False):
    from concourse.bass_utils import run_bass_kernel_spmd

    impl = os.environ.get("GNN_KERNEL_IMPL", "packed")
    in_maps = None
    if impl == "packed":
        in_maps = _host_prep_packed(inputs)
        if in_maps is None:
            impl = "dp8"
    if impl == "dp8":
        in_maps = _host_prep_dp8(inputs)
    if impl not in _NC_CACHE:
        _NC_CACHE[impl] = build_nc_packed() if impl == "packed" else build_nc_dp8()
    nc = _NC_CACHE[impl]
    res = run_bass_kernel_spmd(nc, in_maps, list(range(NCORES)), trace=trace)
    out = np.concatenate([res.results[c]["out"] for c in range(NCORES)], axis=0)
    return out.astype(np.float32), res


def kernel(**inputs) -> np.ndarray:
    out, _ = run(inputs)
    return out
